# revision 13
# baseline (speedup 1.0000x reference)
"""Anisotropic upsampling kernel for Trainium2 (8 NeuronCores, batch-sharded).

Computes, for inputs x0 (8,64,64,256), x1 (8,64,128,128), x2 (8,64,256,64):
    out0 = (up_h(x0) + up_w(x1)) / 2   -> (8,64,128,256)
    out1 = (up_h(x1) + up_w(x2)) / 2   -> (8,64,256,128)
where up() is the stride-2, length-5 normalized zero-insert upsampler:
    up(x)[2m]   = (x[m-1]+x[m]+x[m+1])/3   (edges: mean of the 2 valid taps)
    up(x)[2m+1] = (x[m]+x[m+1])/2          (edge m=W-1: x[W-1])

Layout: partitions p = 2*c + ha where ha selects the top/bottom half of the
h range.  Every DRAM transfer is a single 128-partition DMA whose OUTER walk
dim is c (64 entries): the SDMA spray gives each of the 16 engines a
contiguous block of 4 channels = 8 partitions = one SBUF port group, so DMA
runs at the HBM line rate with one contiguous descriptor per partition.

Both up_h and up_w are free-axis stencils computed in bf16 so VectorE's
2x packing mode applies (h-shifts are row-pitch multiples, always 4B
aligned; the w-branch +1 shift is made aligned by a GpSimd shifted copy).
VectorE does the tap sums and the fused scale-accumulate merges into the
fp32 output tile; ScalarE does the f32->bf16 casts and the scaled
row-parity copies.  Global h-edge rows use per-partition scale vectors
(ha parity) plus two 1-row halo DMAs per output.
"""

import numpy as np

_NC_CACHE = {}


def _build():
    import concourse.bass as bass
    import concourse.mybir as mybir
    from concourse import bacc
    from concourse.tile import TileContext

    f32 = mybir.dt.float32
    bf16 = mybir.dt.bfloat16
    MUL = mybir.AluOpType.mult
    ADD = mybir.AluOpType.add

    nc = bacc.Bacc("TRN2", target_bir_lowering=False, debug=False, num_devices=8)

    xs = {
        "x0": nc.dram_tensor("x0", [64, 64, 256], f32, kind="ExternalInput"),
        "x1": nc.dram_tensor("x1", [64, 128, 128], f32, kind="ExternalInput"),
        "x2": nc.dram_tensor("x2", [64, 256, 64], f32, kind="ExternalInput"),
    }
    out0 = nc.dram_tensor("out0", [64, 128, 256], f32, kind="ExternalOutput")
    out1 = nc.dram_tensor("out1", [64, 256, 128], f32, kind="ExternalOutput")

    # per-partition (ha-parity) scale vectors for the h-edge fixups
    ha = (np.arange(128) % 2).astype(np.float32)  # p = 2c + ha
    consts = np.stack(
        [0.25 * (1 - ha), (1.0 / 6.0) * ha, (1.0 / 6.0) * (1 - ha), 0.25 * ha],
        axis=1,
    )  # (128, 4)

    def walk(d, shape, lo, n):
        """c-outer (ha c | rows lo..lo+n, w) 4D walk of DRAM d (64, HH, W).
        lo is a within-half row index; partition order p = 2c + ha."""
        C, HH, W = shape
        return bass.AP(
            d, lo * W,
            [[HH * W, C], [(HH // 2) * W, 2], [W, n], [1, W]],
        )

    def halo_row(d, shape, row0, row1):
        """1-row 128-partition walk: ha=0 partitions read row0, ha=1 read
        row1 (global row indices; row1-row0 is the ha stride)."""
        C, HH, W = shape
        return bass.AP(
            d, row0 * W,
            [[HH * W, C], [(row1 - row0) * W, 2], [1, W]],
        )

    with TileContext(nc) as tc:
        with (
            tc.tile_pool(name="cpool", bufs=1) as cpool,
            tc.tile_pool(name="inpool", bufs=3) as inpool,
            tc.tile_pool(name="stpool", bufs=3) as stpool,
            tc.tile_pool(name="opool", bufs=2) as opool,
        ):
            cv_d = nc.inline_tensor(consts, "edge_scales")
            cv = cpool.tile([128, 4], f32, tag="cv")
            nc.sync.dma_start(out=cv, in_=cv_d[:, :])

            def emit_chunk(out_d, out_shape, xv_d, xv_shape, xh_d, xh_shape,
                           H, W, R, i):
                """One R-row chunk of out = 0.5*up_h(xv) + 0.5*up_w(xh).

                out_d: (64, 2H, 2W); xv_d: (64, H, 2W); xh_d: (64, 2H, W).
                Partition p = 2c+ha covers output rows h2 = H*ha + r.
                """
                n_chunks = H // R
                nh = R // 2
                r0 = i * R
                m0 = r0 // 2  # within-half first source row

                XV = inpool.tile([128, nh + 2, 2 * W], f32, tag="xv")
                if i == 0:
                    # rows j=1..nh+1 <- within-half rows [0, nh+1);
                    # halo j=0: ha=1 reads global row H/2-1, ha=0 gets a
                    # junk-but-finite row (killed by 0-scale fixup)
                    nc.sync.dma_start(
                        out=XV[:, 1:nh + 2, :],
                        in_=walk(xv_d, xv_shape, 0, nh + 1))
                    nc.sync.dma_start(
                        out=XV[:, 0, :],
                        in_=halo_row(xv_d, xv_shape, 0, H // 2 - 1))
                elif i == n_chunks - 1:
                    # rows j=0..nh <- within-half rows [m0-1, m0+nh);
                    # j=nh+1: ha=0 reads global row H/2 (real cross-half
                    # halo), ha=1 re-reads its row H-1 so SH[nh] doubles
                    # the last tap (odd-edge trick)
                    nc.sync.dma_start(
                        out=XV[:, 0:nh + 1, :],
                        in_=walk(xv_d, xv_shape, m0 - 1, nh + 1))
                    nc.sync.dma_start(
                        out=XV[:, nh + 1, :],
                        in_=halo_row(xv_d, xv_shape, m0 + nh, H - 1))
                else:
                    nc.sync.dma_start(
                        out=XV, in_=walk(xv_d, xv_shape, m0 - 1, nh + 2))
                XH = inpool.tile([128, R, W], f32, tag="xh")
                nc.sync.dma_start(
                    out=XH, in_=walk(xh_d, xh_shape, r0, R))

                # ---- bf16 casts (ScalarE, 2x via packed writes) ----
                XVb = inpool.tile([128, nh + 2, 2 * W], bf16, tag="xvb")
                nc.scalar.copy(XVb, XV)
                XHb = inpool.tile([128, R, W], bf16, tag="xhb")
                nc.scalar.copy(XHb, XH)
                # +1-shifted copy so the w-branch adds are 4B-aligned
                XHs = inpool.tile([128, R, W - 1], bf16, tag="xhs")
                nc.gpsimd.tensor_copy(XHs, XHb[:, :, 1:W])

                # ---- h-branch stencil sums (VectorE bf16 2x) ----
                SH = stpool.tile([128, nh + 1, 2 * W], bf16, tag="sh")
                nc.vector.tensor_add(
                    SH, XVb[:, 0:nh + 1, :], XVb[:, 1:nh + 2, :])
                TH = stpool.tile([128, nh, 2 * W], bf16, tag="th")
                nc.vector.tensor_add(
                    TH, SH[:, 0:nh, :], XVb[:, 2:nh + 2, :])

                # ---- w-branch stencil sums (VectorE bf16 2x) ----
                # SWE cols 0..W-2 = s_w; col W-1 = 2*x[W-1] (odd edge)
                SWE = stpool.tile([128, R, W], bf16, tag="swe")
                nc.vector.tensor_add(
                    SWE[:, :, 0:W - 1], XHb[:, :, 0:W - 1], XHs)
                nc.scalar.mul(
                    SWE[:, :, W - 1:W], XHb[:, :, W - 1:W], 2.0)
                # TW dense t_w[m], m=1..W-2 stored at cols 0..W-3
                TW = stpool.tile([128, R, W - 2], bf16, tag="tw")
                nc.vector.tensor_add(
                    TW, SWE[:, :, 0:W - 2], XHb[:, :, 2:W])

                # ---- h-branch scaled row-parity writes (ScalarE) ----
                O = opool.tile([128, R, 2 * W], f32, tag="o")
                nc.scalar.mul(O[:, 1:R:2, :], SH[:, 1:nh + 1, :], 0.25)
                nc.scalar.mul(O[:, 0:R:2, :], TH, 1.0 / 6.0)

                # ---- global h-edge row fixups (per-partition scales) ----
                if i == 0:
                    nc.scalar.mul(O[:, 0, :], SH[:, 1, :], cv[:, 0:1])
                    nc.vector.scalar_tensor_tensor(
                        O[:, 0, :], TH[:, 0, :], cv[:, 1:2], O[:, 0, :],
                        op0=MUL, op1=ADD)
                if i == n_chunks - 1:
                    nc.scalar.mul(
                        O[:, R - 2, :], TH[:, nh - 1, :], cv[:, 2:3])
                    nc.vector.scalar_tensor_tensor(
                        O[:, R - 2, :], SH[:, nh - 1, :], cv[:, 3:4],
                        O[:, R - 2, :], op0=MUL, op1=ADD)
                    # row R-1 needs no fixup (doubled-tap trick)

                # ---- w-plane fused scale-accumulate merges (VectorE) ----
                nc.vector.scalar_tensor_tensor(
                    O[:, :, 1:2 * W:2], SWE, 0.25, O[:, :, 1:2 * W:2],
                    op0=MUL, op1=ADD)
                nc.vector.scalar_tensor_tensor(
                    O[:, :, 2:2 * W - 2:2], TW, 1.0 / 6.0,
                    O[:, :, 2:2 * W - 2:2], op0=MUL, op1=ADD)
                nc.vector.scalar_tensor_tensor(
                    O[:, :, 0:2 * W - 1:2 * W - 2],
                    SWE[:, :, 0:W - 1:W - 2], 0.25,
                    O[:, :, 0:2 * W - 1:2 * W - 2], op0=MUL, op1=ADD)

                nc.sync.dma_start(
                    out=walk(out_d, out_shape, r0, R), in_=O)

            # interleave out0 and out1 chunks so loads/compute of the two
            # output pipelines overlap
            for i in range(4):
                emit_chunk(out0, (64, 128, 256), xs["x0"], (64, 64, 256),
                           xs["x1"], (64, 128, 128), H=64, W=128, R=16, i=i)
                emit_chunk(out1, (64, 256, 128), xs["x1"], (64, 128, 128),
                           xs["x2"], (64, 256, 64), H=128, W=64, R=32, i=i)

    nc.compile()
    return nc


def _get_nc():
    if "nc" not in _NC_CACHE:
        _NC_CACHE["nc"] = _build()
    return _NC_CACHE["nc"]


def kernel(x0, x1, x2):
    from concourse.bass_utils import run_bass_kernel_spmd

    nc = _get_nc()
    in_maps = [
        {
            "x0": np.ascontiguousarray(x0[b]),
            "x1": np.ascontiguousarray(x1[b]),
            "x2": np.ascontiguousarray(x2[b]),
        }
        for b in range(8)
    ]
    res = run_bass_kernel_spmd(nc, in_maps, core_ids=list(range(8)))
    o0 = np.stack([res.results[b]["out0"] for b in range(8)])
    o1 = np.stack([res.results[b]["out1"] for b in range(8)])
    return o0, o1


# revision 14
# speedup vs baseline: 1.4854x; 1.4854x over previous
"""Anisotropic upsampling kernel for Trainium2 (8 NeuronCores, batch-sharded).

Computes, for inputs x0 (8,64,64,256), x1 (8,64,128,128), x2 (8,64,256,64):
    out0 = (up_h(x0) + up_w(x1)) / 2   -> (8,64,128,256)
    out1 = (up_h(x1) + up_w(x2)) / 2   -> (8,64,256,128)
where up() is the stride-2, length-5 normalized zero-insert upsampler:
    up(x)[2m]   = (x[m-1]+x[m]+x[m+1])/3   (edges: mean of the 2 valid taps)
    up(x)[2m+1] = (x[m]+x[m+1])/2          (edge m=W-1: x[W-1])

Layout: partitions p = 2*c + ha where ha selects the top/bottom half of the
h range.  Every DRAM transfer is a single 128-partition DMA whose OUTER walk
dim is c (64 entries): the SDMA spray gives each of the 16 engines a
contiguous block of 4 channels = 8 partitions = one SBUF port group, so DMA
runs at the HBM line rate with one contiguous descriptor per partition.

Both up_h and up_w are free-axis stencils computed in bf16 so VectorE's
2x packing mode applies (h-shifts are row-pitch multiples, always 4B
aligned; the w-branch +1 shift is made aligned by a GpSimd shifted copy).
VectorE does the tap sums and the fused scale-accumulate merges into the
fp32 output tile; ScalarE does the f32->bf16 casts and the scaled
row-parity copies.  Global h-edge rows use per-partition scale vectors
(ha parity) plus two 1-row halo DMAs per output.
"""

import numpy as np

_NC_CACHE = {}


def _build():
    import concourse.bass as bass
    import concourse.mybir as mybir
    from concourse import bacc
    from concourse.tile import TileContext

    f32 = mybir.dt.float32
    bf16 = mybir.dt.bfloat16
    MUL = mybir.AluOpType.mult
    ADD = mybir.AluOpType.add

    nc = bacc.Bacc("TRN2", target_bir_lowering=False, debug=False, num_devices=8)

    xs = {
        "x0": nc.dram_tensor("x0", [64, 64, 256], f32, kind="ExternalInput"),
        "x1": nc.dram_tensor("x1", [64, 128, 128], f32, kind="ExternalInput"),
        "x2": nc.dram_tensor("x2", [64, 256, 64], f32, kind="ExternalInput"),
    }
    out0 = nc.dram_tensor("out0", [64, 128, 256], f32, kind="ExternalOutput")
    out1 = nc.dram_tensor("out1", [64, 256, 128], f32, kind="ExternalOutput")

    # per-partition (ha-parity) scale vectors for the h-edge fixups
    ha = (np.arange(128) % 2).astype(np.float32)  # p = 2c + ha
    consts = np.stack(
        [0.25 * (1 - ha), (1.0 / 6.0) * ha, (1.0 / 6.0) * (1 - ha), 0.25 * ha],
        axis=1,
    )  # (128, 4)

    def walk(d, shape, lo, n):
        """c-outer (ha c | rows lo..lo+n, w) 4D walk of DRAM d (64, HH, W).
        lo is a within-half row index; partition order p = 2c + ha."""
        C, HH, W = shape
        return bass.AP(
            d, lo * W,
            [[HH * W, C], [(HH // 2) * W, 2], [W, n], [1, W]],
        )

    def halo_row(d, shape, row0, row1):
        """1-row 128-partition walk: ha=0 partitions read row0, ha=1 read
        row1 (global row indices; row1-row0 is the ha stride)."""
        C, HH, W = shape
        return bass.AP(
            d, row0 * W,
            [[HH * W, C], [(row1 - row0) * W, 2], [1, W]],
        )

    with TileContext(nc) as tc:
        with (
            tc.tile_pool(name="cpool", bufs=1) as cpool,
            tc.tile_pool(name="inpool", bufs=3) as inpool,
            tc.tile_pool(name="stpool", bufs=3) as stpool,
            tc.tile_pool(name="opool", bufs=2) as opool,
        ):
            cv_d = nc.inline_tensor(consts, "edge_scales")
            cv = cpool.tile([128, 4], f32, tag="cv")
            nc.sync.dma_start(out=cv, in_=cv_d[:, :])

            def emit_chunk(out_d, out_shape, xv_d, xv_shape, xh_d, xh_shape,
                           H, W, R, i):
                """One R-row chunk of out = 0.5*up_h(xv) + 0.5*up_w(xh).

                out_d: (64, 2H, 2W); xv_d: (64, H, 2W); xh_d: (64, 2H, W).
                Partition p = 2c+ha covers output rows h2 = H*ha + r.
                """
                n_chunks = H // R
                nh = R // 2
                r0 = i * R
                m0 = r0 // 2  # within-half first source row

                XV = inpool.tile([128, nh + 2, 2 * W], f32, tag="xv")
                if i == 0:
                    # rows j=1..nh+1 <- within-half rows [0, nh+1);
                    # halo j=0: ha=1 reads global row H/2-1, ha=0 gets a
                    # junk-but-finite row (killed by 0-scale fixup)
                    nc.sync.dma_start(
                        out=XV[:, 1:nh + 2, :],
                        in_=walk(xv_d, xv_shape, 0, nh + 1))
                    nc.sync.dma_start(
                        out=XV[:, 0, :],
                        in_=halo_row(xv_d, xv_shape, 0, H // 2 - 1))
                elif i == n_chunks - 1:
                    # rows j=0..nh <- within-half rows [m0-1, m0+nh);
                    # j=nh+1: ha=0 reads global row H/2 (real cross-half
                    # halo), ha=1 re-reads its row H-1 so SH[nh] doubles
                    # the last tap (odd-edge trick)
                    nc.sync.dma_start(
                        out=XV[:, 0:nh + 1, :],
                        in_=walk(xv_d, xv_shape, m0 - 1, nh + 1))
                    nc.sync.dma_start(
                        out=XV[:, nh + 1, :],
                        in_=halo_row(xv_d, xv_shape, m0 + nh, H - 1))
                else:
                    nc.sync.dma_start(
                        out=XV, in_=walk(xv_d, xv_shape, m0 - 1, nh + 2))
                XH = inpool.tile([128, R, W], f32, tag="xh")
                nc.sync.dma_start(
                    out=XH, in_=walk(xh_d, xh_shape, r0, R))

                # ---- bf16 casts (ScalarE, 2x via packed writes) ----
                XVb = inpool.tile([128, nh + 2, 2 * W], bf16, tag="xvb")
                nc.scalar.copy(XVb, XV)
                XHb = inpool.tile([128, R, W], bf16, tag="xhb")
                nc.scalar.copy(XHb, XH)
                # +1-shifted cast (from the f32 source, which is always
                # 4B-aligned) so the w-branch bf16 adds run packed
                XHs = inpool.tile([128, R, W - 1], bf16, tag="xhs")
                nc.scalar.copy(XHs, XH[:, :, 1:W])

                # ---- h-branch stencil sums (VectorE bf16 2x) ----
                SH = stpool.tile([128, nh + 1, 2 * W], bf16, tag="sh")
                nc.vector.tensor_add(
                    SH, XVb[:, 0:nh + 1, :], XVb[:, 1:nh + 2, :])
                TH = stpool.tile([128, nh, 2 * W], bf16, tag="th")
                nc.vector.tensor_add(
                    TH, SH[:, 0:nh, :], XVb[:, 2:nh + 2, :])

                # ---- w-branch stencil sums (VectorE bf16 2x) ----
                # SWE cols 0..W-2 = s_w; col W-1 = 2*x[W-1] (odd edge)
                SWE = stpool.tile([128, R, W], bf16, tag="swe")
                nc.vector.tensor_add(
                    SWE[:, :, 0:W - 1], XHb[:, :, 0:W - 1], XHs)
                nc.scalar.mul(
                    SWE[:, :, W - 1:W], XHb[:, :, W - 1:W], 2.0)
                # TW dense t_w[m], m=1..W-2 stored at cols 0..W-3
                TW = stpool.tile([128, R, W - 2], bf16, tag="tw")
                nc.vector.tensor_add(
                    TW, SWE[:, :, 0:W - 2], XHb[:, :, 2:W])

                # ---- h-branch scaled row-parity writes (ScalarE) ----
                O = opool.tile([128, R, 2 * W], f32, tag="o")
                nc.scalar.mul(O[:, 1:R:2, :], SH[:, 1:nh + 1, :], 0.25)
                nc.scalar.mul(O[:, 0:R:2, :], TH, 1.0 / 6.0)

                # ---- global h-edge row fixups (per-partition scales) ----
                if i == 0:
                    nc.scalar.mul(O[:, 0, :], SH[:, 1, :], cv[:, 0:1])
                    nc.vector.scalar_tensor_tensor(
                        O[:, 0, :], TH[:, 0, :], cv[:, 1:2], O[:, 0, :],
                        op0=MUL, op1=ADD)
                if i == n_chunks - 1:
                    nc.scalar.mul(
                        O[:, R - 2, :], TH[:, nh - 1, :], cv[:, 2:3])
                    nc.vector.scalar_tensor_tensor(
                        O[:, R - 2, :], SH[:, nh - 1, :], cv[:, 3:4],
                        O[:, R - 2, :], op0=MUL, op1=ADD)
                    # row R-1 needs no fixup (doubled-tap trick)

                # ---- w-plane fused scale-accumulate merges (VectorE) ----
                nc.vector.scalar_tensor_tensor(
                    O[:, :, 1:2 * W:2], SWE, 0.25, O[:, :, 1:2 * W:2],
                    op0=MUL, op1=ADD)
                nc.vector.scalar_tensor_tensor(
                    O[:, :, 2:2 * W - 2:2], TW, 1.0 / 6.0,
                    O[:, :, 2:2 * W - 2:2], op0=MUL, op1=ADD)
                nc.vector.scalar_tensor_tensor(
                    O[:, :, 0:2 * W - 1:2 * W - 2],
                    SWE[:, :, 0:W - 1:W - 2], 0.25,
                    O[:, :, 0:2 * W - 1:2 * W - 2], op0=MUL, op1=ADD)

                nc.sync.dma_start(
                    out=walk(out_d, out_shape, r0, R), in_=O)

            # interleave out0 and out1 chunks so loads/compute of the two
            # output pipelines overlap
            for i in range(4):
                emit_chunk(out0, (64, 128, 256), xs["x0"], (64, 64, 256),
                           xs["x1"], (64, 128, 128), H=64, W=128, R=16, i=i)
                emit_chunk(out1, (64, 256, 128), xs["x1"], (64, 128, 128),
                           xs["x2"], (64, 256, 64), H=128, W=64, R=32, i=i)

    nc.compile()
    return nc


def _get_nc():
    if "nc" not in _NC_CACHE:
        _NC_CACHE["nc"] = _build()
    return _NC_CACHE["nc"]


def kernel(x0, x1, x2):
    from concourse.bass_utils import run_bass_kernel_spmd

    nc = _get_nc()
    in_maps = [
        {
            "x0": np.ascontiguousarray(x0[b]),
            "x1": np.ascontiguousarray(x1[b]),
            "x2": np.ascontiguousarray(x2[b]),
        }
        for b in range(8)
    ]
    res = run_bass_kernel_spmd(nc, in_maps, core_ids=list(range(8)))
    o0 = np.stack([res.results[b]["out0"] for b in range(8)])
    o1 = np.stack([res.results[b]["out1"] for b in range(8)])
    return o0, o1


# revision 15
# speedup vs baseline: 1.6811x; 1.1318x over previous
"""Anisotropic upsampling kernel for Trainium2 (8 NeuronCores, batch-sharded).

Computes, for inputs x0 (8,64,64,256), x1 (8,64,128,128), x2 (8,64,256,64):
    out0 = (up_h(x0) + up_w(x1)) / 2   -> (8,64,128,256)
    out1 = (up_h(x1) + up_w(x2)) / 2   -> (8,64,256,128)
where up() is the stride-2, length-5 normalized zero-insert upsampler:
    up(x)[2m]   = (x[m-1]+x[m]+x[m+1])/3   (edges: mean of the 2 valid taps)
    up(x)[2m+1] = (x[m]+x[m+1])/2          (edge m=W-1: x[W-1])

Layout: partitions p = 2*c + ha where ha selects the top/bottom half of the
h range.  Every DRAM transfer is a single 128-partition DMA whose OUTER walk
dim is c (64 entries): the SDMA spray gives each of the 16 engines a
contiguous block of 4 channels = 8 partitions = one SBUF port group, so DMA
runs at the HBM line rate with one contiguous descriptor per partition.

Both up_h and up_w are free-axis stencils computed in bf16 so VectorE's
2x packing mode applies (h-shifts are row-pitch multiples, always 4B
aligned; the w-branch +1 shift is made aligned by a GpSimd shifted copy).
VectorE does the tap sums and the fused scale-accumulate merges into the
fp32 output tile; ScalarE does the f32->bf16 casts and the scaled
row-parity copies.  Global h-edge rows use per-partition scale vectors
(ha parity) plus two 1-row halo DMAs per output.
"""

import numpy as np

_NC_CACHE = {}


def _build():
    import concourse.bass as bass
    import concourse.mybir as mybir
    from concourse import bacc
    from concourse.tile import TileContext

    f32 = mybir.dt.float32
    bf16 = mybir.dt.bfloat16
    MUL = mybir.AluOpType.mult
    ADD = mybir.AluOpType.add

    nc = bacc.Bacc("TRN2", target_bir_lowering=False, debug=False, num_devices=8)

    xs = {
        "x0": nc.dram_tensor("x0", [64, 64, 256], f32, kind="ExternalInput"),
        "x1": nc.dram_tensor("x1", [64, 128, 128], f32, kind="ExternalInput"),
        "x2": nc.dram_tensor("x2", [64, 256, 64], f32, kind="ExternalInput"),
    }
    out0 = nc.dram_tensor("out0", [64, 128, 256], f32, kind="ExternalOutput")
    out1 = nc.dram_tensor("out1", [64, 256, 128], f32, kind="ExternalOutput")

    # per-partition (ha-parity) scale vectors for the h-edge fixups
    ha = (np.arange(128) % 2).astype(np.float32)  # p = 2c + ha
    consts = np.stack(
        [0.25 * (1 - ha), (1.0 / 6.0) * ha, (1.0 / 6.0) * (1 - ha), 0.25 * ha],
        axis=1,
    )  # (128, 4)

    def walk(d, shape, lo, n):
        """c-outer (ha c | rows lo..lo+n, w) 4D walk of DRAM d (64, HH, W).
        lo is a within-half row index; partition order p = 2c + ha."""
        C, HH, W = shape
        return bass.AP(
            d, lo * W,
            [[HH * W, C], [(HH // 2) * W, 2], [W, n], [1, W]],
        )

    def halo_row(d, shape, row0, row1):
        """1-row 128-partition walk: ha=0 partitions read row0, ha=1 read
        row1 (global row indices; row1-row0 is the ha stride)."""
        C, HH, W = shape
        return bass.AP(
            d, row0 * W,
            [[HH * W, C], [(row1 - row0) * W, 2], [1, W]],
        )

    with TileContext(nc) as tc:
        with (
            tc.tile_pool(name="cpool", bufs=1) as cpool,
            tc.tile_pool(name="inpool", bufs=4) as inpool,
            tc.tile_pool(name="stpool", bufs=3) as stpool,
            tc.tile_pool(name="opool", bufs=2) as opool,
        ):
            cv_d = nc.inline_tensor(consts, "edge_scales")
            cv = cpool.tile([128, 4], f32, tag="cv")
            nc.sync.dma_start(out=cv, in_=cv_d[:, :])

            def load_chunk(out_d, out_shape, xv_d, xv_shape, xh_d, xh_shape,
                           H, W, R, i):
                """Issue the input DMAs for one chunk; returns the tiles."""
                n_chunks = H // R
                nh = R // 2
                r0 = i * R
                m0 = r0 // 2  # within-half first source row

                XV = inpool.tile([128, nh + 2, 2 * W], f32, tag="xv")
                if i == 0:
                    # rows j=1..nh+1 <- within-half rows [0, nh+1);
                    # halo j=0: ha=1 reads global row H/2-1, ha=0 gets a
                    # junk-but-finite row (killed by 0-scale fixup)
                    nc.sync.dma_start(
                        out=XV[:, 1:nh + 2, :],
                        in_=walk(xv_d, xv_shape, 0, nh + 1))
                    nc.sync.dma_start(
                        out=XV[:, 0, :],
                        in_=halo_row(xv_d, xv_shape, 0, H // 2 - 1))
                elif i == n_chunks - 1:
                    # rows j=0..nh <- within-half rows [m0-1, m0+nh);
                    # j=nh+1: ha=0 reads global row H/2 (real cross-half
                    # halo), ha=1 re-reads its row H-1 so SH[nh] doubles
                    # the last tap (odd-edge trick)
                    nc.sync.dma_start(
                        out=XV[:, 0:nh + 1, :],
                        in_=walk(xv_d, xv_shape, m0 - 1, nh + 1))
                    nc.sync.dma_start(
                        out=XV[:, nh + 1, :],
                        in_=halo_row(xv_d, xv_shape, m0 + nh, H - 1))
                else:
                    nc.sync.dma_start(
                        out=XV, in_=walk(xv_d, xv_shape, m0 - 1, nh + 2))
                XH = inpool.tile([128, R, W], f32, tag="xh")
                nc.sync.dma_start(
                    out=XH, in_=walk(xh_d, xh_shape, r0, R))
                return XV, XH

            def compute_chunk(out_d, out_shape, xv_d, xv_shape, xh_d,
                              xh_shape, H, W, R, i, XV, XH):
                """Compute + store one chunk given its loaded tiles."""
                n_chunks = H // R
                nh = R // 2
                r0 = i * R

                # ---- bf16 casts (ScalarE, 2x via packed writes) ----
                XVb = inpool.tile([128, nh + 2, 2 * W], bf16, tag="xvb")
                nc.scalar.copy(XVb, XV)
                XHb = inpool.tile([128, R, W], bf16, tag="xhb")
                nc.scalar.copy(XHb, XH)
                # +1-shifted cast (from the f32 source, which is always
                # 4B-aligned) so the w-branch bf16 adds run packed
                XHs = inpool.tile([128, R, W - 1], bf16, tag="xhs")
                nc.scalar.copy(XHs, XH[:, :, 1:W])

                # ---- h-branch stencil sums (VectorE bf16 2x) ----
                SH = stpool.tile([128, nh + 1, 2 * W], bf16, tag="sh")
                nc.vector.tensor_add(
                    SH, XVb[:, 0:nh + 1, :], XVb[:, 1:nh + 2, :])
                TH = stpool.tile([128, nh, 2 * W], bf16, tag="th")
                nc.vector.tensor_add(
                    TH, SH[:, 0:nh, :], XVb[:, 2:nh + 2, :])

                # ---- w-branch stencil sums (VectorE bf16 2x) ----
                # SWE cols 0..W-2 = s_w; col W-1 = 2*x[W-1] (odd edge)
                SWE = stpool.tile([128, R, W], bf16, tag="swe")
                nc.vector.tensor_add(
                    SWE[:, :, 0:W - 1], XHb[:, :, 0:W - 1], XHs)
                nc.scalar.mul(
                    SWE[:, :, W - 1:W], XHb[:, :, W - 1:W], 2.0)
                # TW dense t_w[m], m=1..W-2 stored at cols 0..W-3
                TW = stpool.tile([128, R, W - 2], bf16, tag="tw")
                nc.vector.tensor_add(
                    TW, SWE[:, :, 0:W - 2], XHb[:, :, 2:W])

                # ---- h-branch scaled row-parity writes (ScalarE) ----
                O = opool.tile([128, R, 2 * W], f32, tag="o")
                nc.scalar.mul(O[:, 1:R:2, :], SH[:, 1:nh + 1, :], 0.25)
                nc.scalar.mul(O[:, 0:R:2, :], TH, 1.0 / 6.0)

                # ---- global h-edge row fixups (per-partition scales) ----
                if i == 0:
                    nc.scalar.mul(O[:, 0, :], SH[:, 1, :], cv[:, 0:1])
                    nc.vector.scalar_tensor_tensor(
                        O[:, 0, :], TH[:, 0, :], cv[:, 1:2], O[:, 0, :],
                        op0=MUL, op1=ADD)
                if i == n_chunks - 1:
                    nc.scalar.mul(
                        O[:, R - 2, :], TH[:, nh - 1, :], cv[:, 2:3])
                    nc.vector.scalar_tensor_tensor(
                        O[:, R - 2, :], SH[:, nh - 1, :], cv[:, 3:4],
                        O[:, R - 2, :], op0=MUL, op1=ADD)
                    # row R-1 needs no fixup (doubled-tap trick)

                # ---- w-plane fused scale-accumulate merges (VectorE) ----
                nc.vector.scalar_tensor_tensor(
                    O[:, :, 1:2 * W:2], SWE, 0.25, O[:, :, 1:2 * W:2],
                    op0=MUL, op1=ADD)
                nc.vector.scalar_tensor_tensor(
                    O[:, :, 2:2 * W - 2:2], TW, 1.0 / 6.0,
                    O[:, :, 2:2 * W - 2:2], op0=MUL, op1=ADD)
                nc.vector.scalar_tensor_tensor(
                    O[:, :, 0:2 * W - 1:2 * W - 2],
                    SWE[:, :, 0:W - 1:W - 2], 0.25,
                    O[:, :, 0:2 * W - 1:2 * W - 2], op0=MUL, op1=ADD)

                nc.sync.dma_start(
                    out=walk(out_d, out_shape, r0, R), in_=O)

            # interleave out0 and out1 chunks, with loads emitted ahead of
            # computes/stores so store waits on the DMA ring don't
            # head-of-line-block the next chunks' loads
            units = []
            for i in range(4):
                units.append((out0, (64, 128, 256), xs["x0"], (64, 64, 256),
                              xs["x1"], (64, 128, 128), 64, 128, 16, i))
                units.append((out1, (64, 256, 128), xs["x1"], (64, 128, 128),
                              xs["x2"], (64, 256, 64), 128, 64, 32, i))
            PREFETCH = 2
            tiles = {}
            for k in range(min(PREFETCH, len(units))):
                tiles[k] = load_chunk(*units[k])
            for k in range(len(units)):
                if k + PREFETCH < len(units):
                    tiles[k + PREFETCH] = load_chunk(*units[k + PREFETCH])
                compute_chunk(*units[k], *tiles.pop(k))

    nc.compile()
    return nc


def _get_nc():
    if "nc" not in _NC_CACHE:
        _NC_CACHE["nc"] = _build()
    return _NC_CACHE["nc"]


def kernel(x0, x1, x2):
    from concourse.bass_utils import run_bass_kernel_spmd

    nc = _get_nc()
    in_maps = [
        {
            "x0": np.ascontiguousarray(x0[b]),
            "x1": np.ascontiguousarray(x1[b]),
            "x2": np.ascontiguousarray(x2[b]),
        }
        for b in range(8)
    ]
    res = run_bass_kernel_spmd(nc, in_maps, core_ids=list(range(8)))
    o0 = np.stack([res.results[b]["out0"] for b in range(8)])
    o1 = np.stack([res.results[b]["out1"] for b in range(8)])
    return o0, o1
